# revision 11
# baseline (speedup 1.0000x reference)
"""Trainium2 Bass kernel for an 8-head cross-attention block (v2).

Math (per reference):
    Q = video @ Wq[h]           [4096, 64]  per head
    K = text  @ Wk[h]           [1024, 64]
    V = text  @ Wv[h]           [1024, 64]
    att = softmax(Q @ K^T)      [4096, 1024]   (no scaling)
    y_h = att @ V               [4096, 64]
    out = concat_h(y_h) @ Wout + pos_enc(4096, 512)

Sharding: head-parallel over 8 NeuronCores; core h owns head h and its 64
rows of Wout (row-parallel) and emits a full [4096, 512] f32 partial that
the host all-reduces (+ positional encoding).

v2 layout/schedule (single fused j-loop over 8 chunks of 512 queries):
  Q-proj (fp16) -> E = K^T.T Q^T (fp16, row-tiled pairs) -> exp on Scalar
  (shift -3.5, writes fp8e4m3) -> att@[V|1] as fp8 DoubleRow matmuls ->
  per-chunk 1/den (reciprocal + gpsimd partition-broadcast, no DRAM
  round-trip) -> y normalized to fp8 -> out-proj as fp8 DoubleRow ->
  direct PSUM->DRAM f32 output DMA (no output cast ops at all).
All phases software-pipeline across j via tile pools; PSUM budget is
e(4)+q(1)+y/o(3) = 8 banks.
"""

import numpy as np
import ml_dtypes

from concourse import bacc
import concourse.mybir as mybir
from concourse.tile import TileContext
from concourse.bass_utils import run_bass_kernel_spmd

N, M, D, H, DH = 4096, 1024, 512, 8, 64
P = 128
NC = 512          # n-chunk width (queries per j)
NJ = N // NC      # 8 chunks
DC = D // P       # 4 contraction chunks of 128
MT = M // P       # 8 key tiles of 128
VW = 80           # padded V' row pitch (65 -> 80 for 16B-aligned DR stride)
F32 = mybir.dt.float32
FP16 = mybir.dt.float16
FP8 = mybir.dt.float8e4
EXP = mybir.ActivationFunctionType.Exp
DR = mybir.MatmulPerfMode.DoubleRow
EXP_SHIFT = -5.0   # exp(E + shift): keeps exp within fp8e4m3(IEEE) range, max 240
NCORES = 8

_CACHE: dict = {}
TRACE = False          # test harness can flip this before calling kernel()
LAST_RESULT = None     # BassKernelResults of the last run (for profiling)


def _body(tc, nc, vT, tT, wqkv, wo, out, dscr):
    with tc.tile_pool(name="const", bufs=1) as cp, \
         tc.tile_pool(name="pp", bufs=2) as pp, \
         tc.tile_pool(name="ps_e", bufs=2, space="PSUM") as pe_pool, \
         tc.tile_pool(name="ps_q", bufs=1, space="PSUM") as pq_pool, \
         tc.tile_pool(name="ps_yo", bufs=3, space="PSUM") as pyo_pool, \
         tc.tile_pool(name="jp", bufs=2) as jp:

        vt_sb = cp.tile([P, DC * N], FP16, tag="vt")     # [128, c*4096]
        tt_sb = cp.tile([P, DC * M], FP16, tag="tt")     # [128, c*1024]
        wqkv_sb = cp.tile([P, DC * 3 * DH], FP16, tag="wqkv")
        wo_sb = cp.tile([P, D], FP16, tag="wo")          # Wout dup on halves
        qt_sb = cp.tile([P, N], FP16, tag="qt")          # Q^T dup on both halves
        kt_sb = cp.tile([P, M], FP16, tag="kt")          # K^T dup on both halves
        v_sb = cp.tile([P, MT * VW], FP8, tag="vsb")     # V'[m, 0:64]=V, [64]=1

        def wslice(which, c):
            base = c * 3 * DH + which * DH
            return wqkv_sb[:, base:base + DH]

        # ---- input DMAs: simple 2D shapes; vT sliced on the gpsimd queue ---
        for c in range(DC):
            nc.sync.dma_start(
                out=tt_sb[:, c * M:(c + 1) * M], in_=tT[c * P:(c + 1) * P, :])
        for c in range(DC):
            nc.sync.dma_start(
                out=wqkv_sb[:, c * 3 * DH:(c + 1) * 3 * DH],
                in_=wqkv[c * P:(c + 1) * P, :])
        nc.sync.dma_start(out=wo_sb[0:DH, :], in_=wo[:, :])
        nc.sync.dma_start(out=wo_sb[DH:P, :], in_=wo[:, :])
        vt3 = vt_sb.rearrange("p (c n) -> p c n", c=DC)
        QB = 1024                                        # vT DMA block (2 chunks)
        for b in range(N // QB):
            for c in range(DC):
                nc.gpsimd.dma_start(
                    out=vt3[:, c, b * QB:(b + 1) * QB],
                    in_=vT[c * P:(c + 1) * P, b * QB:(b + 1) * QB])

        v3 = v_sb.rearrange("p (m e) -> p m e", e=VW)    # [128, 8, 80]
        nc.vector.memset(v3[:, :, DH], 1.0)
        bias_sb = cp.tile([P, 1], F32, tag="bias")
        nc.vector.memset(bias_sb[:, :], EXP_SHIFT)

        # ---- K-proj into kt (dup halves); V-proj into v3 (fp8) ------------
        for half in range(2):
            k_ps = pq_pool.tile([DH, NC], F32, tag="q")
            for c in range(DC):
                nc.tensor.matmul(
                    k_ps[:, :],
                    wslice(1, c),
                    tt_sb[:, c * M + half * NC: c * M + (half + 1) * NC],
                    start=(c == 0), stop=(c == DC - 1))
            sl = slice(half * NC, (half + 1) * NC)
            nc.vector.tensor_copy(out=kt_sb[0:DH, sl], in_=k_ps[:, :])
            nc.vector.tensor_copy(out=kt_sb[DH:P, sl], in_=k_ps[:, :])
        for mt in range(MT):
            v_ps = pyo_pool.tile([P, DH], F32, tag="yo")
            for c in range(DC):
                nc.tensor.matmul(
                    v_ps[:, :],
                    tt_sb[:, c * M + mt * P: c * M + (mt + 1) * P],
                    wslice(2, c),
                    start=(c == 0), stop=(c == DC - 1))
            nc.vector.tensor_copy(out=v3[:, mt, 0:DH], in_=v_ps[:, :])

        # ---- fused per-chunk pipeline (j in pairs for col-tiled Q-proj) ----
        for j2 in range(NJ // 2):
            # Q-proj for both chunks of the pair: even chunk lands on PSUM
            # partitions 0:64, odd on 64:128 (2x column tiling), interleaved
            # so the two streams overlap on the PE array.
            q_ps = pq_pool.tile([P, NC], F32, tag="q")
            for c in range(DC):
                for dj in range(2):
                    j = j2 * 2 + dj
                    nc.tensor.matmul(
                        q_ps[dj * DH:(dj + 1) * DH, :],
                        wslice(0, c),
                        vt3[:, c, j * NC:(j + 1) * NC],
                        start=(c == 0), stop=(c == DC - 1))
            for dj in range(2):
                j = j2 * 2 + dj
                jsl = slice(j * NC, (j + 1) * NC)
                hsl = slice(dj * DH, (dj + 1) * DH)
                nc.vector.tensor_copy(out=qt_sb[0:DH, jsl], in_=q_ps[hsl, :])
                nc.vector.tensor_copy(out=qt_sb[DH:P, jsl], in_=q_ps[hsl, :])

            for dj in range(2):
                j = j2 * 2 + dj
                jsl = slice(j * NC, (j + 1) * NC)

                # E tiles (row-tiled fp16 pairs) -> exp -> P^T (fp8)
                pt = pp.tile([P, MT * NC], FP8, tag="p")
                for pr in range(MT // 2):
                    mt = pr * 2
                    e_ps = pe_pool.tile([P, 2 * NC], F32, tag="e")
                    nc.tensor.matmul(
                        e_ps[:, 0:NC],
                        kt_sb[0:DH, mt * P:(mt + 1) * P],
                        qt_sb[0:DH, jsl],
                        start=True, stop=True)
                    nc.tensor.matmul(
                        e_ps[:, NC:2 * NC],
                        kt_sb[DH:P, (mt + 1) * P:(mt + 2) * P],
                        qt_sb[DH:P, jsl],
                        start=True, stop=True)
                    nc.scalar.activation(
                        pt[:, pr * 2 * NC:(pr + 1) * 2 * NC], e_ps[:, :],
                        EXP, bias=bias_sb[:, :])

                # Y' = [V|1]^T @ P^T via fp8 DoubleRow, accumulating over pairs
                y_ps = pyo_pool.tile([DH + 1, NC], F32, tag="yo")
                for pr in range(MT // 2):
                    nc.tensor.matmul(
                        y_ps[:, :],
                        v3[:, pr * 2:pr * 2 + 2, 0:DH + 1],
                        pt.rearrange("p (pr i n) -> p pr i n", pr=MT // 2, i=2)[:, pr],
                        start=(pr == 0), stop=(pr == MT // 2 - 1),
                        perf_mode=DR)

                # denominator: row -> DRAM -> [128, 4] scatter -> 1/x
                # (runs concurrently with y16/out-proj; only gates the casts)
                den_sb = jp.tile([1, NC], F32, tag="den")
                rsrc = jp.tile([P, NC // P], F32, tag="rsrc")
                rc_sb = jp.tile([P, NC // P], F32, tag="rc")
                nc.vector.tensor_copy(out=den_sb[:, :], in_=y_ps[DH:DH + 1, :])
                nc.sync.dma_start(out=dscr[jsl], in_=den_sb[:, :])
                nc.sync.dma_start(
                    out=rsrc[:, :],
                    in_=dscr[jsl].rearrange("(t p) -> p t", p=P))
                nc.vector.reciprocal(rc_sb[:, :], rsrc[:, :])

                # unnormalized Y^T dup on halves (fp16)
                y16 = jp.tile([P, NC], FP16, tag="y16")
                nc.vector.tensor_copy(out=y16[0:DH, :], in_=y_ps[0:DH, :])
                nc.vector.tensor_copy(out=y16[DH:P, :], in_=y_ps[0:DH, :])

                # out-proj: row-tiled K=64 pairs (two token tiles stream
                # concurrently); normalize+cast fused, split vector/scalar
                ot = jp.tile([P, (NC // P) * D], FP16, tag="ot")
                for t in range(0, NC // P, 2):
                    o_a = pyo_pool.tile([P, D], F32, tag="yo")
                    o_b = pyo_pool.tile([P, D], F32, tag="yo")
                    nc.tensor.matmul(
                        o_a[:, :],
                        y16[0:DH, t * P:(t + 1) * P],
                        wo_sb[0:DH, :],
                        start=True, stop=True)
                    nc.tensor.matmul(
                        o_b[:, :],
                        y16[DH:P, (t + 1) * P:(t + 2) * P],
                        wo_sb[DH:P, :],
                        start=True, stop=True)
                    nc.vector.tensor_scalar_mul(
                        ot[:, t * D:(t + 1) * D], o_a[:, :], rc_sb[:, t:t + 1])
                    nc.scalar.activation(
                        ot[:, (t + 1) * D:(t + 2) * D], o_b[:, :],
                        mybir.ActivationFunctionType.Copy,
                        scale=rc_sb[:, t + 1:t + 2])
                nc.gpsimd.dma_start(
                    out=out.rearrange("(j t p) d -> p j t d", p=P, t=NC // P)[:, j],
                    in_=ot.rearrange("p (t d) -> p t d", t=NC // P))


def _build():
    nc = bacc.Bacc("TRN2", target_bir_lowering=False, debug=False)
    vT = nc.dram_tensor("vT", [D, N], FP16, kind="ExternalInput")
    tT = nc.dram_tensor("tT", [D, M], FP16, kind="ExternalInput")
    wqkv = nc.dram_tensor("wqkv", [D, 3 * DH], FP16, kind="ExternalInput")
    wo = nc.dram_tensor("wo", [DH, D], FP16, kind="ExternalInput")
    out = nc.dram_tensor("out", [N, D], FP16, kind="ExternalOutput")
    dscr = nc.dram_tensor("dscr", [N], F32)
    with TileContext(nc) as tc:
        _body(tc, nc, vT[:, :], tT[:, :], wqkv[:, :],
              wo[:, :], out[:, :], dscr[:])
    nc.compile()
    return nc


def _pos_encoding():
    # Mirror the reference's jnp ops bit-for-bit (numpy's f32 sin/exp differ
    # by enough ULPs to dominate the error budget at pos/freq ~ 4e3).
    import jax
    import jax.numpy as jnp
    with jax.default_device(jax.devices("cpu")[0]):
        pos = jnp.arange(N, dtype=jnp.float32)
        freq = jnp.exp(
            (jnp.arange(D // 2, dtype=jnp.float32) / D)
            * jnp.log(jnp.float32(10000.0)))
        x = pos[:, None] / freq
        pe = jnp.stack((jnp.sin(x), jnp.cos(x)), axis=-1)
        return np.asarray(pe.reshape(N, D), dtype=np.float32)


def _fp16(a):
    return np.ascontiguousarray(np.asarray(a, dtype=np.float32).astype(np.float16))


def _fp8(a):
    return np.ascontiguousarray(
        np.asarray(a, dtype=np.float32).astype(ml_dtypes.float8_e4m3))


def kernel(video_features, text_features, Wq, Wk, Wv, Wout):
    global LAST_RESULT
    if "nc" not in _CACHE:
        _CACHE["nc"] = _build()
        _CACHE["pe"] = _pos_encoding()
    nc = _CACHE["nc"]

    vT = _fp16(np.asarray(video_features, dtype=np.float32).T)
    tT = _fp16(np.asarray(text_features, dtype=np.float32).T)
    Wq = np.asarray(Wq, dtype=np.float32)
    Wk = np.asarray(Wk, dtype=np.float32)
    Wv = np.asarray(Wv, dtype=np.float32)
    Wout = np.asarray(Wout, dtype=np.float32)

    in_maps = []
    for h in range(NCORES):
        in_maps.append({
            "vT": vT,
            "tT": tT,
            "wqkv": _fp16(np.concatenate([Wq[h], Wk[h], Wv[h]], axis=1)),
            "wo": _fp16(Wout[h * DH:(h + 1) * DH, :]),
        })
    res = run_bass_kernel_spmd(nc, in_maps, list(range(NCORES)), trace=TRACE)
    LAST_RESULT = res
    acc = res.results[0]["out"].astype(np.float32)
    for h in range(1, NCORES):
        acc = acc + res.results[h]["out"].astype(np.float32)
    return (acc + _CACHE["pe"]).astype(np.float32)


# revision 12
# speedup vs baseline: 1.1574x; 1.1574x over previous
"""Trainium2 Bass kernel for an 8-head cross-attention block (v2).

Math (per reference):
    Q = video @ Wq[h]           [4096, 64]  per head
    K = text  @ Wk[h]           [1024, 64]
    V = text  @ Wv[h]           [1024, 64]
    att = softmax(Q @ K^T)      [4096, 1024]   (no scaling)
    y_h = att @ V               [4096, 64]
    out = concat_h(y_h) @ Wout + pos_enc(4096, 512)

Sharding: head-parallel over 8 NeuronCores; core h owns head h and its 64
rows of Wout (row-parallel) and emits a full [4096, 512] f32 partial that
the host all-reduces (+ positional encoding).

v2 layout/schedule (single fused j-loop over 8 chunks of 512 queries):
  Q-proj (fp16) -> E = K^T.T Q^T (fp16, row-tiled pairs) -> exp on Scalar
  (shift -3.5, writes fp8e4m3) -> att@[V|1] as fp8 DoubleRow matmuls ->
  per-chunk 1/den (reciprocal + gpsimd partition-broadcast, no DRAM
  round-trip) -> y normalized to fp8 -> out-proj as fp8 DoubleRow ->
  direct PSUM->DRAM f32 output DMA (no output cast ops at all).
All phases software-pipeline across j via tile pools; PSUM budget is
e(4)+q(1)+y/o(3) = 8 banks.
"""

import numpy as np
import ml_dtypes

from concourse import bacc
import concourse.mybir as mybir
from concourse.tile import TileContext
from concourse.bass_utils import run_bass_kernel_spmd

N, M, D, H, DH = 4096, 1024, 512, 8, 64
P = 128
NC = 512          # n-chunk width (queries per j)
NJ = N // NC      # 8 chunks
DC = D // P       # 4 contraction chunks of 128
MT = M // P       # 8 key tiles of 128
VW = 80           # padded V' row pitch (65 -> 80 for 16B-aligned DR stride)
F32 = mybir.dt.float32
FP16 = mybir.dt.float16
FP8 = mybir.dt.float8e4
EXP = mybir.ActivationFunctionType.Exp
DR = mybir.MatmulPerfMode.DoubleRow
EXP_SHIFT = -5.0   # exp(E + shift): keeps exp within fp8e4m3(IEEE) range, max 240
NCORES = 8

_CACHE: dict = {}
TRACE = False          # test harness can flip this before calling kernel()
LAST_RESULT = None     # BassKernelResults of the last run (for profiling)


def _body(tc, nc, vT, tT, wqkv, wo, out):
    with tc.tile_pool(name="const", bufs=1) as cp, \
         tc.tile_pool(name="pp", bufs=2) as pp, \
         tc.tile_pool(name="ps_e", bufs=2, space="PSUM") as pe_pool, \
         tc.tile_pool(name="ps_q", bufs=1, space="PSUM") as pq_pool, \
         tc.tile_pool(name="ps_yo", bufs=3, space="PSUM") as pyo_pool, \
         tc.tile_pool(name="jp", bufs=2) as jp:

        vt_sb = cp.tile([P, DC * N], FP16, tag="vt")     # [128, c*4096]
        tt_sb = cp.tile([P, DC * M], FP16, tag="tt")     # [128, c*1024]
        wqkv_sb = cp.tile([P, DC * 3 * DH], FP16, tag="wqkv")
        wo_sb = cp.tile([P, D], FP16, tag="wo")          # Wout dup on halves
        qt_sb = cp.tile([P, N], FP16, tag="qt")          # Q^T dup on both halves
        kt_sb = cp.tile([P, M], FP16, tag="kt")          # K^T dup on both halves
        v_sb = cp.tile([P, MT * VW], FP8, tag="vsb")     # V'[m, 0:64]=V, [64]=1

        def wslice(which, c):
            base = c * 3 * DH + which * DH
            return wqkv_sb[:, base:base + DH]

        # ---- input DMAs: weights first, tt per (c, half) so K-proj h0 can
        # start early; wo last (first needed ~20us in); vT on gpsimd queue --
        for c in range(DC):
            nc.sync.dma_start(
                out=wqkv_sb[:, c * 3 * DH:(c + 1) * 3 * DH],
                in_=wqkv[c * P:(c + 1) * P, :])
        for half in range(2):
            for c in range(DC):
                nc.sync.dma_start(
                    out=tt_sb[:, c * M + half * NC: c * M + (half + 1) * NC],
                    in_=tT[c * P:(c + 1) * P, half * NC:(half + 1) * NC])
        nc.sync.dma_start(out=wo_sb[0:DH, :], in_=wo[:, :])
        nc.sync.dma_start(out=wo_sb[DH:P, :], in_=wo[:, :])
        vt3 = vt_sb.rearrange("p (c n) -> p c n", c=DC)
        QB = 1024                                        # vT DMA block (2 chunks)
        for b in range(N // QB):
            for c in range(DC):
                nc.gpsimd.dma_start(
                    out=vt3[:, c, b * QB:(b + 1) * QB],
                    in_=vT[c * P:(c + 1) * P, b * QB:(b + 1) * QB])

        v3 = v_sb.rearrange("p (m e) -> p m e", e=VW)    # [128, 8, 80]
        nc.vector.memset(v3[:, :, DH], 1.0)
        bias_sb = cp.tile([P, 1], F32, tag="bias")
        nc.vector.memset(bias_sb[:, :], EXP_SHIFT)
        one_sb = cp.tile([1, 1], F32, tag="one")
        nc.vector.memset(one_sb[:, :], 1.0)

        # ---- K-proj into kt (dup halves); V-proj into v3 (fp8) ------------
        for half in range(2):
            k_ps = pq_pool.tile([DH, NC], F32, tag="q")
            for c in range(DC):
                nc.tensor.matmul(
                    k_ps[:, :],
                    wslice(1, c),
                    tt_sb[:, c * M + half * NC: c * M + (half + 1) * NC],
                    start=(c == 0), stop=(c == DC - 1))
            sl = slice(half * NC, (half + 1) * NC)
            nc.vector.tensor_copy(out=kt_sb[0:DH, sl], in_=k_ps[:, :])
            nc.vector.tensor_copy(out=kt_sb[DH:P, sl], in_=k_ps[:, :])
            for mt in range(half * (MT // 2), (half + 1) * (MT // 2)):
                v_ps = pyo_pool.tile([P, DH], F32, tag="yo")
                for c in range(DC):
                    nc.tensor.matmul(
                        v_ps[:, :],
                        tt_sb[:, c * M + mt * P: c * M + (mt + 1) * P],
                        wslice(2, c),
                        start=(c == 0), stop=(c == DC - 1))
                nc.vector.tensor_copy(out=v3[:, mt, 0:DH], in_=v_ps[:, :])

        # ---- fused per-chunk pipeline (j in pairs for col-tiled Q-proj) ----
        for j2 in range(NJ // 2):
            # Q-proj for both chunks of the pair: even chunk lands on PSUM
            # partitions 0:64, odd on 64:128 (2x column tiling), interleaved
            # so the two streams overlap on the PE array.
            q_ps = pq_pool.tile([P, NC], F32, tag="q")
            for c in range(DC):
                for dj in range(2):
                    j = j2 * 2 + dj
                    nc.tensor.matmul(
                        q_ps[dj * DH:(dj + 1) * DH, :],
                        wslice(0, c),
                        vt3[:, c, j * NC:(j + 1) * NC],
                        start=(c == 0), stop=(c == DC - 1))
            for dj in range(2):
                j = j2 * 2 + dj
                jsl = slice(j * NC, (j + 1) * NC)
                hsl = slice(dj * DH, (dj + 1) * DH)
                nc.vector.tensor_copy(out=qt_sb[0:DH, jsl], in_=q_ps[hsl, :])
                nc.vector.tensor_copy(out=qt_sb[DH:P, jsl], in_=q_ps[hsl, :])

            for dj in range(2):
                j = j2 * 2 + dj
                jsl = slice(j * NC, (j + 1) * NC)

                # E tiles (row-tiled fp16 pairs) -> exp -> P^T (fp8)
                pt = pp.tile([P, MT * NC], FP8, tag="p")
                for pr in range(MT // 2):
                    mt = pr * 2
                    e_ps = pe_pool.tile([P, 2 * NC], F32, tag="e")
                    nc.tensor.matmul(
                        e_ps[:, 0:NC],
                        kt_sb[0:DH, mt * P:(mt + 1) * P],
                        qt_sb[0:DH, jsl],
                        start=True, stop=True)
                    nc.tensor.matmul(
                        e_ps[:, NC:2 * NC],
                        kt_sb[DH:P, (mt + 1) * P:(mt + 2) * P],
                        qt_sb[DH:P, jsl],
                        start=True, stop=True)
                    nc.scalar.activation(
                        pt[:, pr * 2 * NC:(pr + 1) * 2 * NC], e_ps[:, :],
                        EXP, bias=bias_sb[:, :])

                # Y' = [V|1]^T @ P^T via fp8 DoubleRow, accumulating over pairs
                y_ps = pyo_pool.tile([DH + 1, NC], F32, tag="yo")
                for pr in range(MT // 2):
                    nc.tensor.matmul(
                        y_ps[:, :],
                        v3[:, pr * 2:pr * 2 + 2, 0:DH + 1],
                        pt.rearrange("p (pr i n) -> p pr i n", pr=MT // 2, i=2)[:, pr],
                        start=(pr == 0), stop=(pr == MT // 2 - 1),
                        perf_mode=DR)

                # denominator: SBUF row -> 4 tiny PE transposes -> [128,4]
                # -> 1/x  (all on-chip; no DRAM round-trip latency)
                den_sb = jp.tile([1, NC], F32, tag="den")
                rc_sb = jp.tile([P, NC // P], F32, tag="rc")
                nc.vector.tensor_copy(out=den_sb[:, :], in_=y_ps[DH:DH + 1, :])
                tr_ps = pyo_pool.tile([P, NC // P], F32, tag="yo")
                for t in range(NC // P):
                    nc.tensor.matmul(
                        tr_ps[:, t:t + 1],
                        den_sb[0:1, t * P:(t + 1) * P],
                        one_sb[0:1, 0:1],
                        start=True, stop=True, is_transpose=True)
                nc.vector.reciprocal(rc_sb[:, :], tr_ps[:, :])

                # unnormalized Y^T dup on halves (fp16)
                y16 = jp.tile([P, NC], FP16, tag="y16")
                nc.vector.tensor_copy(out=y16[0:DH, :], in_=y_ps[0:DH, :])
                nc.vector.tensor_copy(out=y16[DH:P, :], in_=y_ps[0:DH, :])

                # out-proj: row-tiled K=64 pairs (two token tiles stream
                # concurrently); normalize+cast fused, split vector/scalar
                ot = jp.tile([P, (NC // P) * D], FP16, tag="ot")
                for t in range(0, NC // P, 2):
                    o_a = pyo_pool.tile([P, D], F32, tag="yo")
                    o_b = pyo_pool.tile([P, D], F32, tag="yo")
                    nc.tensor.matmul(
                        o_a[:, :],
                        y16[0:DH, t * P:(t + 1) * P],
                        wo_sb[0:DH, :],
                        start=True, stop=True)
                    nc.tensor.matmul(
                        o_b[:, :],
                        y16[DH:P, (t + 1) * P:(t + 2) * P],
                        wo_sb[DH:P, :],
                        start=True, stop=True)
                    nc.vector.tensor_scalar_mul(
                        ot[:, t * D:(t + 1) * D], o_a[:, :], rc_sb[:, t:t + 1])
                    nc.scalar.activation(
                        ot[:, (t + 1) * D:(t + 2) * D], o_b[:, :],
                        mybir.ActivationFunctionType.Copy,
                        scale=rc_sb[:, t + 1:t + 2])
                nc.gpsimd.dma_start(
                    out=out.rearrange("(j t p) d -> p j t d", p=P, t=NC // P)[:, j],
                    in_=ot.rearrange("p (t d) -> p t d", t=NC // P))


def _build():
    nc = bacc.Bacc("TRN2", target_bir_lowering=False, debug=False)
    vT = nc.dram_tensor("vT", [D, N], FP16, kind="ExternalInput")
    tT = nc.dram_tensor("tT", [D, M], FP16, kind="ExternalInput")
    wqkv = nc.dram_tensor("wqkv", [D, 3 * DH], FP16, kind="ExternalInput")
    wo = nc.dram_tensor("wo", [DH, D], FP16, kind="ExternalInput")
    out = nc.dram_tensor("out", [N, D], FP16, kind="ExternalOutput")
    with TileContext(nc) as tc:
        _body(tc, nc, vT[:, :], tT[:, :], wqkv[:, :],
              wo[:, :], out[:, :])
    nc.compile()
    return nc


def _pos_encoding():
    # Mirror the reference's jnp ops bit-for-bit (numpy's f32 sin/exp differ
    # by enough ULPs to dominate the error budget at pos/freq ~ 4e3).
    import jax
    import jax.numpy as jnp
    with jax.default_device(jax.devices("cpu")[0]):
        pos = jnp.arange(N, dtype=jnp.float32)
        freq = jnp.exp(
            (jnp.arange(D // 2, dtype=jnp.float32) / D)
            * jnp.log(jnp.float32(10000.0)))
        x = pos[:, None] / freq
        pe = jnp.stack((jnp.sin(x), jnp.cos(x)), axis=-1)
        return np.asarray(pe.reshape(N, D), dtype=np.float32)


def _fp16(a):
    return np.ascontiguousarray(np.asarray(a, dtype=np.float32).astype(np.float16))


def _fp8(a):
    return np.ascontiguousarray(
        np.asarray(a, dtype=np.float32).astype(ml_dtypes.float8_e4m3))


def kernel(video_features, text_features, Wq, Wk, Wv, Wout):
    global LAST_RESULT
    if "nc" not in _CACHE:
        _CACHE["nc"] = _build()
        _CACHE["pe"] = _pos_encoding()
    nc = _CACHE["nc"]

    vT = _fp16(np.asarray(video_features, dtype=np.float32).T)
    tT = _fp16(np.asarray(text_features, dtype=np.float32).T)
    Wq = np.asarray(Wq, dtype=np.float32)
    Wk = np.asarray(Wk, dtype=np.float32)
    Wv = np.asarray(Wv, dtype=np.float32)
    Wout = np.asarray(Wout, dtype=np.float32)

    in_maps = []
    for h in range(NCORES):
        in_maps.append({
            "vT": vT,
            "tT": tT,
            "wqkv": _fp16(np.concatenate([Wq[h], Wk[h], Wv[h]], axis=1)),
            "wo": _fp16(Wout[h * DH:(h + 1) * DH, :]),
        })
    res = run_bass_kernel_spmd(nc, in_maps, list(range(NCORES)), trace=TRACE)
    LAST_RESULT = res
    acc = res.results[0]["out"].astype(np.float32)
    for h in range(1, NCORES):
        acc = acc + res.results[h]["out"].astype(np.float32)
    return (acc + _CACHE["pe"]).astype(np.float32)


# revision 13
# speedup vs baseline: 1.4562x; 1.2581x over previous
"""Trainium2 Bass kernel for an 8-head cross-attention block (v2).

Math (per reference):
    Q = video @ Wq[h]           [4096, 64]  per head
    K = text  @ Wk[h]           [1024, 64]
    V = text  @ Wv[h]           [1024, 64]
    att = softmax(Q @ K^T)      [4096, 1024]   (no scaling)
    y_h = att @ V               [4096, 64]
    out = concat_h(y_h) @ Wout + pos_enc(4096, 512)

Sharding: head-parallel over 8 NeuronCores; core h owns head h and its 64
rows of Wout (row-parallel) and emits a full [4096, 512] f32 partial that
the host all-reduces (+ positional encoding).

v2 layout/schedule (single fused j-loop over 8 chunks of 512 queries):
  Q-proj (fp16) -> E = K^T.T Q^T (fp16, row-tiled pairs) -> exp on Scalar
  (shift -3.5, writes fp8e4m3) -> att@[V|1] as fp8 DoubleRow matmuls ->
  per-chunk 1/den (reciprocal + gpsimd partition-broadcast, no DRAM
  round-trip) -> y normalized to fp8 -> out-proj as fp8 DoubleRow ->
  direct PSUM->DRAM f32 output DMA (no output cast ops at all).
All phases software-pipeline across j via tile pools; PSUM budget is
e(4)+q(1)+y/o(3) = 8 banks.
"""

import numpy as np
import ml_dtypes

from concourse import bacc
import concourse.mybir as mybir
from concourse.tile import TileContext
from concourse.bass_utils import run_bass_kernel_spmd

N, M, D, H, DH = 4096, 1024, 512, 8, 64
P = 128
NC = 512          # n-chunk width (queries per j)
NJ = N // NC      # 8 chunks
DC = D // P       # 4 contraction chunks of 128
MT = M // P       # 8 key tiles of 128
VW = 80           # padded V' row pitch (65 -> 80 for 16B-aligned DR stride)
F32 = mybir.dt.float32
FP16 = mybir.dt.float16
FP8 = mybir.dt.float8e4
EXP = mybir.ActivationFunctionType.Exp
DR = mybir.MatmulPerfMode.DoubleRow
EXP_SHIFT = -5.0   # exp(E + shift): keeps exp within fp8e4m3(IEEE) range, max 240
NCORES = 8

_CACHE: dict = {}
TRACE = False          # test harness can flip this before calling kernel()
LAST_RESULT = None     # BassKernelResults of the last run (for profiling)


def _body(tc, nc, vT, tT, wqkv, wo, out):
    with tc.tile_pool(name="const", bufs=1) as cp, \
         tc.tile_pool(name="pp", bufs=2) as pp, \
         tc.tile_pool(name="ps_e", bufs=2, space="PSUM") as pe_pool, \
         tc.tile_pool(name="ps_q", bufs=1, space="PSUM") as pq_pool, \
         tc.tile_pool(name="ps_yo", bufs=3, space="PSUM") as pyo_pool, \
         tc.tile_pool(name="jp", bufs=2) as jp:

        vt_sb = cp.tile([P, DC * N], FP16, tag="vt")     # [128, c*4096]
        tt_sb = cp.tile([P, DC * M], FP16, tag="tt")     # [128, c*1024]
        wqkv_sb = cp.tile([P, DC * 3 * DH], FP16, tag="wqkv")
        wo_sb = cp.tile([P, D], FP16, tag="wo")          # Wout dup on halves
        qt_sb = cp.tile([P, N], FP16, tag="qt")          # Q^T dup on both halves
        kt_sb = cp.tile([P, M], FP16, tag="kt")          # K^T dup on both halves
        v_sb = cp.tile([P, MT * VW], FP8, tag="vsb")     # V'[m, 0:64]=V, [64]=1

        def wslice(which, c):
            base = c * 3 * DH + which * DH
            return wqkv_sb[:, base:base + DH]

        # ---- input DMAs, all on the sync ring in consumption order so the
        # round-robin DMA engines fill SBUF in the order compute needs it:
        # weights -> tt half0 -> vT block0 -> tt half1 -> vT block1 -> wo ->
        # remaining vT. Output DMAs use the gpsimd ring.
        vt3 = vt_sb.rearrange("p (c n) -> p c n", c=DC)
        QB = 1024                                        # vT DMA block (2 chunks)

        def vt_block(b):
            for c in range(DC):
                nc.sync.dma_start(
                    out=vt3[:, c, b * QB:(b + 1) * QB],
                    in_=vT[c * P:(c + 1) * P, b * QB:(b + 1) * QB])

        for c in range(DC):
            nc.sync.dma_start(
                out=wqkv_sb[:, c * 3 * DH:(c + 1) * 3 * DH],
                in_=wqkv[c * P:(c + 1) * P, :])
        for c in range(DC):
            nc.sync.dma_start(
                out=tt_sb[:, c * M: c * M + NC],
                in_=tT[c * P:(c + 1) * P, 0:NC])
        vt_block(0)
        for c in range(DC):
            nc.sync.dma_start(
                out=tt_sb[:, c * M + NC: c * M + 2 * NC],
                in_=tT[c * P:(c + 1) * P, NC:2 * NC])
        vt_block(1)
        nc.sync.dma_start(out=wo_sb[0:DH, :], in_=wo[:, :])
        nc.sync.dma_start(out=wo_sb[DH:P, :], in_=wo[:, :])
        vt_block(2)
        vt_block(3)

        v3 = v_sb.rearrange("p (m e) -> p m e", e=VW)    # [128, 8, 80]
        nc.vector.memset(v3[:, :, DH], 1.0)
        bias_sb = cp.tile([P, 1], F32, tag="bias")
        nc.vector.memset(bias_sb[:, :], EXP_SHIFT)
        one_sb = cp.tile([1, 1], F32, tag="one")
        nc.vector.memset(one_sb[:, :], 1.0)

        # ---- K-proj into kt (dup halves); V-proj into v3 (fp8) ------------
        for half in range(2):
            k_ps = pq_pool.tile([DH, NC], F32, tag="q")
            for c in range(DC):
                nc.tensor.matmul(
                    k_ps[:, :],
                    wslice(1, c),
                    tt_sb[:, c * M + half * NC: c * M + (half + 1) * NC],
                    start=(c == 0), stop=(c == DC - 1))
            sl = slice(half * NC, (half + 1) * NC)
            nc.vector.tensor_copy(out=kt_sb[0:DH, sl], in_=k_ps[:, :])
            nc.vector.tensor_copy(out=kt_sb[DH:P, sl], in_=k_ps[:, :])
            for mt in range(half * (MT // 2), (half + 1) * (MT // 2)):
                v_ps = pyo_pool.tile([P, DH], F32, tag="yo")
                for c in range(DC):
                    nc.tensor.matmul(
                        v_ps[:, :],
                        tt_sb[:, c * M + mt * P: c * M + (mt + 1) * P],
                        wslice(2, c),
                        start=(c == 0), stop=(c == DC - 1))
                nc.vector.tensor_copy(out=v3[:, mt, 0:DH], in_=v_ps[:, :])

        # ---- fused per-chunk pipeline (j in pairs for col-tiled Q-proj) ----
        for j2 in range(NJ // 2):
            # Q-proj for both chunks of the pair: even chunk lands on PSUM
            # partitions 0:64, odd on 64:128 (2x column tiling), interleaved
            # so the two streams overlap on the PE array.
            q_ps = pq_pool.tile([P, NC], F32, tag="q")
            for c in range(DC):
                for dj in range(2):
                    j = j2 * 2 + dj
                    nc.tensor.matmul(
                        q_ps[dj * DH:(dj + 1) * DH, :],
                        wslice(0, c),
                        vt3[:, c, j * NC:(j + 1) * NC],
                        start=(c == 0), stop=(c == DC - 1))
            for dj in range(2):
                j = j2 * 2 + dj
                jsl = slice(j * NC, (j + 1) * NC)
                hsl = slice(dj * DH, (dj + 1) * DH)
                nc.vector.tensor_copy(out=qt_sb[0:DH, jsl], in_=q_ps[hsl, :])
                nc.vector.tensor_copy(out=qt_sb[DH:P, jsl], in_=q_ps[hsl, :])

            for dj in range(2):
                j = j2 * 2 + dj
                jsl = slice(j * NC, (j + 1) * NC)

                # E tiles (row-tiled fp16 pairs) -> exp -> P^T (fp8)
                pt = pp.tile([P, MT * NC], FP8, tag="p")
                for pr in range(MT // 2):
                    mt = pr * 2
                    e_ps = pe_pool.tile([P, 2 * NC], F32, tag="e")
                    nc.tensor.matmul(
                        e_ps[:, 0:NC],
                        kt_sb[0:DH, mt * P:(mt + 1) * P],
                        qt_sb[0:DH, jsl],
                        start=True, stop=True)
                    nc.tensor.matmul(
                        e_ps[:, NC:2 * NC],
                        kt_sb[DH:P, (mt + 1) * P:(mt + 2) * P],
                        qt_sb[DH:P, jsl],
                        start=True, stop=True)
                    nc.scalar.activation(
                        pt[:, pr * 2 * NC:(pr + 1) * 2 * NC], e_ps[:, :],
                        EXP, bias=bias_sb[:, :])

                # Y' = [V|1]^T @ P^T via fp8 DoubleRow, accumulating over pairs
                y_ps = pyo_pool.tile([DH + 1, NC], F32, tag="yo")
                for pr in range(MT // 2):
                    nc.tensor.matmul(
                        y_ps[:, :],
                        v3[:, pr * 2:pr * 2 + 2, 0:DH + 1],
                        pt.rearrange("p (pr i n) -> p pr i n", pr=MT // 2, i=2)[:, pr],
                        start=(pr == 0), stop=(pr == MT // 2 - 1),
                        perf_mode=DR)

                # denominator: SBUF row -> 4 tiny PE transposes -> [128,4]
                # -> 1/x  (all on-chip; no DRAM round-trip latency)
                den_sb = jp.tile([1, NC], F32, tag="den")
                rc_sb = jp.tile([P, NC // P], F32, tag="rc")
                nc.vector.tensor_copy(out=den_sb[:, :], in_=y_ps[DH:DH + 1, :])
                tr_ps = pyo_pool.tile([P, NC // P], F32, tag="yo")
                for t in range(NC // P):
                    nc.tensor.matmul(
                        tr_ps[:, t:t + 1],
                        den_sb[0:1, t * P:(t + 1) * P],
                        one_sb[0:1, 0:1],
                        start=True, stop=True, is_transpose=True)
                nc.vector.reciprocal(rc_sb[:, :], tr_ps[:, :])

                # unnormalized Y^T dup on halves (fp16)
                y16 = jp.tile([P, NC], FP16, tag="y16")
                nc.vector.tensor_copy(out=y16[0:DH, :], in_=y_ps[0:DH, :])
                nc.vector.tensor_copy(out=y16[DH:P, :], in_=y_ps[0:DH, :])

                # out-proj: row-tiled K=64 pairs (two token tiles stream
                # concurrently); normalize+cast fused, split vector/scalar
                ot = jp.tile([P, (NC // P) * D], FP16, tag="ot")
                for t in range(0, NC // P, 2):
                    o_a = pyo_pool.tile([P, D], F32, tag="yo")
                    o_b = pyo_pool.tile([P, D], F32, tag="yo")
                    nc.tensor.matmul(
                        o_a[:, :],
                        y16[0:DH, t * P:(t + 1) * P],
                        wo_sb[0:DH, :],
                        start=True, stop=True)
                    nc.tensor.matmul(
                        o_b[:, :],
                        y16[DH:P, (t + 1) * P:(t + 2) * P],
                        wo_sb[DH:P, :],
                        start=True, stop=True)
                    nc.vector.tensor_scalar_mul(
                        ot[:, t * D:(t + 1) * D], o_a[:, :], rc_sb[:, t:t + 1])
                    nc.scalar.activation(
                        ot[:, (t + 1) * D:(t + 2) * D], o_b[:, :],
                        mybir.ActivationFunctionType.Copy,
                        scale=rc_sb[:, t + 1:t + 2])
                outr = out.rearrange("(j t p) d -> p j t d", p=P, t=NC // P)
                otr = ot.rearrange("p (t d) -> p t d", t=NC // P)
                nc.gpsimd.dma_start(out=outr[:, j, 0:2], in_=otr[:, 0:2])
                nc.gpsimd.dma_start(out=outr[:, j, 2:4], in_=otr[:, 2:4])


def _build():
    nc = bacc.Bacc("TRN2", target_bir_lowering=False, debug=False)
    vT = nc.dram_tensor("vT", [D, N], FP16, kind="ExternalInput")
    tT = nc.dram_tensor("tT", [D, M], FP16, kind="ExternalInput")
    wqkv = nc.dram_tensor("wqkv", [D, 3 * DH], FP16, kind="ExternalInput")
    wo = nc.dram_tensor("wo", [DH, D], FP16, kind="ExternalInput")
    out = nc.dram_tensor("out", [N, D], FP16, kind="ExternalOutput")
    with TileContext(nc) as tc:
        _body(tc, nc, vT[:, :], tT[:, :], wqkv[:, :],
              wo[:, :], out[:, :])
    nc.compile()
    return nc


def _pos_encoding():
    # Mirror the reference's jnp ops bit-for-bit (numpy's f32 sin/exp differ
    # by enough ULPs to dominate the error budget at pos/freq ~ 4e3).
    import jax
    import jax.numpy as jnp
    with jax.default_device(jax.devices("cpu")[0]):
        pos = jnp.arange(N, dtype=jnp.float32)
        freq = jnp.exp(
            (jnp.arange(D // 2, dtype=jnp.float32) / D)
            * jnp.log(jnp.float32(10000.0)))
        x = pos[:, None] / freq
        pe = jnp.stack((jnp.sin(x), jnp.cos(x)), axis=-1)
        return np.asarray(pe.reshape(N, D), dtype=np.float32)


def _fp16(a):
    return np.ascontiguousarray(np.asarray(a, dtype=np.float32).astype(np.float16))


def _fp8(a):
    return np.ascontiguousarray(
        np.asarray(a, dtype=np.float32).astype(ml_dtypes.float8_e4m3))


def kernel(video_features, text_features, Wq, Wk, Wv, Wout):
    global LAST_RESULT
    if "nc" not in _CACHE:
        _CACHE["nc"] = _build()
        _CACHE["pe"] = _pos_encoding()
    nc = _CACHE["nc"]

    vT = _fp16(np.asarray(video_features, dtype=np.float32).T)
    tT = _fp16(np.asarray(text_features, dtype=np.float32).T)
    Wq = np.asarray(Wq, dtype=np.float32)
    Wk = np.asarray(Wk, dtype=np.float32)
    Wv = np.asarray(Wv, dtype=np.float32)
    Wout = np.asarray(Wout, dtype=np.float32)

    in_maps = []
    for h in range(NCORES):
        in_maps.append({
            "vT": vT,
            "tT": tT,
            "wqkv": _fp16(np.concatenate([Wq[h], Wk[h], Wv[h]], axis=1)),
            "wo": _fp16(Wout[h * DH:(h + 1) * DH, :]),
        })
    res = run_bass_kernel_spmd(nc, in_maps, list(range(NCORES)), trace=TRACE)
    LAST_RESULT = res
    acc = res.results[0]["out"].astype(np.float32)
    for h in range(1, NCORES):
        acc = acc + res.results[h]["out"].astype(np.float32)
    return (acc + _CACHE["pe"]).astype(np.float32)
